# revision 7
# baseline (speedup 1.0000x reference)
"""Trainium2 Bass kernel for CrossDecoder kNN-mining margin loss.

fp8-DoubleRow mining with exact index embedding:

Per query q and candidate y, rank by T(q,y) = 18*(q3.y3)_{data dims}
- 32*m_y + j/2048 where q3 = 2*round(3q), y3 = round(3y) (exact small
integers in fp8-e4m3), m_y = round((9|y|^2-4608)/32) (full-|y|^2 bias,
base-16 signed digits in two extra contraction rows), and j/2048 is
the candidate's index within its 1888-wide max8 group, encoded in four
extra rows (base-8 digits e/64 against weights 2^-5/2^-2/2/16 — all
exact e4m3 normals). Every product and partial sum is an exact
multiple of 2^-11 with magnitude < 2^13, so the fp32 PSUM value is
EXACT: integer part = mining score, fraction decodes the candidate
index. Two HW caveats (probed): the DoubleRow per-cell pair-adder
drops a tiny fraction product paired with a large data product, so the
aux rows' pair-partner slots are zeroed; and the self-match row
reaches ~+9100 > 2^13 and loses its last fraction bit, so the host
injects self explicitly. The host recomputes exact fp64 distances for
the 128 kept candidates per query, so mining noise (sigma ~7 from the
12 sacrificed dims) only affects WHICH near-ties are kept, never the
values used in the loss. Simulated end-to-end rel err ~3e-6; measured
on HW 6.5e-6.

Device (SPMD 8 cores, candidates sharded 3750/core, padded 3776):
queries are deduplicated (~5440 unique of 6000) and packed into
128-query tiles. Per query-tile and half-shard (4 chunks of 472 in 4
PSUM banks, one [128, 4, 512] fp32 tile), 2 pass-major DoubleRow
matmuls per chunk (K_eff=256 each, moving operand 944 fp8 elems at
2/cycle), then ONE DVE max8 over the 1888 live columns. Top-8 per
half-shard * 2 halves * 8 cores = 128 kept values per unique query.
"""

import os
import numpy as np

M_, N_, D_, T_ = 2, 30000, 256, 3000
KD = M_ * D_                   # 512
NCORES = 8
NSHARD = N_ // NCORES          # 3750
FCH = 470                      # candidate chunk width (even)
NFC = 8                        # chunks per shard
NPAD = FCH * NFC               # 3776
HGRP = 4                       # chunks per max8 group (4 PSUM banks)
GRP = HGRP * FCH               # 1888 candidates per max8
PASSES = int(os.environ.get("KNN_PASSES", "2"))
NSLOT = 256 * PASSES
DIMS = NSLOT - 6               # data slots (6 aux; 6 more zeroed partners)
QT = 128                       # queries per tile (PSUM partition dim)
QBLK = 4                       # query tiles per DMA block
KEPT = 16                      # kept values per (query, core)

_cache = {}


def _build_program(nqt_total):
    import concourse.bass as bass
    import concourse.tile as tile
    from concourse import bacc, mybir

    dt = mybir.dt
    DR = mybir.MatmulPerfMode.DoubleRow
    nc = bacc.Bacc(
        "TRN2", target_bir_lowering=False, debug=False, num_devices=NCORES
    )

    nblk = -(-nqt_total // QBLK)
    G2 = 2 * PASSES
    nq = nqt_total * QT
    xq_d = nc.dram_tensor("xq", [128, G2, nq], dt.float8e4, kind="ExternalInput")
    xs_d = nc.dram_tensor("xs", [128, G2, NPAD], dt.float8e4, kind="ExternalInput")
    cand_d = nc.dram_tensor("cand", [nblk, 128, QBLK * KEPT], dt.float32,
                            kind="ExternalOutput")

    with tile.TileContext(nc) as tc:
        with (
            tc.tile_pool(name="resident", bufs=1) as res_pool,
            tc.tile_pool(name="xq", bufs=3) as xq_pool,
            tc.tile_pool(name="cand", bufs=2) as cand_pool,
            tc.tile_pool(name="psum", bufs=2, space=bass.MemorySpace.PSUM) as psum_pool,
        ):
            xs_sb = res_pool.tile([128, G2, NPAD], dt.float8e4, tag="xs")
            # split the resident candidate load so the first matmuls
            # (pass 0, half-shard 0) start after a quarter of the bytes
            for g0 in range(0, G2, 2):
                for c0 in range(0, NPAD, GRP):
                    nc.sync.dma_start(
                        out=xs_sb[:, g0:g0 + 2, c0:c0 + GRP],
                        in_=xs_d[:, g0:g0 + 2, c0:c0 + GRP])

            for blk in range(nblk):
                q0 = blk * QBLK * QT
                nqt = min(QBLK, nqt_total - blk * QBLK)
                xq_sb = xq_pool.tile([128, G2, nqt * QT], dt.float8e4, tag="xq")
                nc.sync.dma_start(out=xq_sb[:, :, :],
                                  in_=xq_d[:, :, q0:q0 + nqt * QT])
                cand_sb = cand_pool.tile([128, nqt * KEPT], dt.float32,
                                         tag="cand")
                for j in range(nqt):
                    for h in range(2):
                        ps = psum_pool.tile([128, HGRP, 512], dt.float32,
                                            tag="ps")
                        for p in range(PASSES):
                            for fi in range(HGRP):
                                f = h * HGRP + fi
                                nc.tensor.matmul(
                                    ps[:, fi, 0:FCH],
                                    lhsT=xq_sb[:, 2 * p:2 * p + 2,
                                               j * QT:(j + 1) * QT],
                                    rhs=xs_sb[:, 2 * p:2 * p + 2,
                                              f * FCH:(f + 1) * FCH],
                                    start=(p == 0), stop=(p == PASSES - 1),
                                    perf_mode=DR,
                                )
                        o = j * KEPT + h * 8
                        nc.vector.max(cand_sb[:, o:o + 8], ps[:, :, 0:FCH])
                nc.sync.dma_start(out=cand_d[blk, :, :nqt * KEPT],
                                  in_=cand_sb[:, :])

    nc.compile()
    return nc


def _get_program(nqt_total):
    key = ("nc", nqt_total)
    if key not in _cache:
        _cache[key] = _build_program(nqt_total)
    return _cache[key]


def _pack_slots(S):
    """S: [NSLOT, n] float32 slot array -> [128, 2*PASSES, n] fp8.

    Slot s maps to (k = s % 128, g = s // 128); pass p contracts slot
    groups g = 2p, 2p+1."""
    import ml_dtypes
    return np.ascontiguousarray(
        S.reshape(2 * PASSES, 128, S.shape[1]).transpose(1, 0, 2)
    ).astype(ml_dtypes.float8_e4m3)


def _prep_inputs(X, uq, nq_pad):
    """X: [N, 512] fp32; uq: unique query indices. Per-core input maps."""
    Y8 = np.clip(np.rint(3.0 * X), -16, 16).astype(np.float32)
    nu = uq.shape[0]
    Sq = np.zeros((NSLOT, nq_pad), np.float32)
    Sq[:DIMS, :nu] = 2.0 * Y8[uq, :DIMS].T
    Sq[DIMS + 0, :nu] = -32.0      # bias digit d0
    Sq[DIMS + 1, :nu] = -32.0      # bias digit 16*d1
    Sq[DIMS + 2, :nu] = 2.0 ** -5  # index e0 -> e0/2048
    Sq[DIMS + 3, :nu] = 2.0 ** -2  # index e1 -> 8*e1/2048
    Sq[DIMS + 4, :nu] = 2.0        # index e2 -> 64*e2/2048
    Sq[DIMS + 5, :nu] = 16.0       # index e3 -> 512*e3/2048
    # the aux rows' DoubleRow pair-partners (same cell, other sub-row)
    # must be zero: the per-cell 2-mult adder drops a tiny fraction
    # product paired with a large data product (HW-probed; the k-chain
    # accumulator itself is exact fp32)
    Sq[NSLOT - 134:NSLOT - 128] = 0.0
    xq = _pack_slots(Sq)

    sqy = (X.astype(np.float64) ** 2).sum(1)
    m = np.clip(np.rint((9.0 * sqy - 4608.0) / 32.0), -136, 136)
    d1 = np.clip(np.rint(m / 16.0), -8, 8)
    d0 = m - 16.0 * d1

    jg = (np.arange(NPAD) % GRP).astype(np.float32)  # index within group
    e0 = (jg % 8) / 64.0
    e1 = ((jg // 8) % 8) / 64.0
    e2 = ((jg // 64) % 8) / 64.0
    e3 = (jg // 512) / 64.0

    per_core = []
    for c in range(NCORES):
        lo = c * NSHARD
        Sy = np.zeros((NSLOT, NPAD), np.float32)
        Sy[:DIMS, :NSHARD] = Y8[lo:lo + NSHARD, :DIMS].T
        Sy[DIMS + 0, :NSHARD] = d0[lo:lo + NSHARD]
        Sy[DIMS + 1, :NSHARD] = 16.0 * d1[lo:lo + NSHARD]
        Sy[DIMS + 0, NSHARD:] = 8.0       # pads rank last: bias -4352
        Sy[DIMS + 1, NSHARD:] = 128.0
        Sy[DIMS + 2] = e0
        Sy[DIMS + 3] = e1
        Sy[DIMS + 4] = e2
        Sy[DIMS + 5] = e3
        Sy[NSLOT - 134:NSLOT - 128] = 0.0  # aux pair-partners (see xq)
        per_core.append({"xq": xq, "xs": _pack_slots(Sy)})
    return per_core


def _mine(in_maps, nqt_total, trace=False):
    from concourse.bass_utils import run_bass_kernel_spmd

    nc = _get_program(nqt_total)
    try:
        res = run_bass_kernel_spmd(nc, in_maps, list(range(NCORES)), trace=trace)
    except Exception:
        if not trace:
            raise
        res = run_bass_kernel_spmd(nc, in_maps, list(range(NCORES)), trace=False)
    _cache["last_result"] = res
    nblk = -(-nqt_total // QBLK)
    cores = []
    for i in range(NCORES):
        c = res.results[i]["cand"].reshape(nblk, 128, QBLK, KEPT)
        cores.append(
            c.transpose(0, 2, 1, 3).reshape(nblk * QBLK * 128, KEPT))
    return np.concatenate(cores, axis=1)   # [nq_pad, 128] mining values


def _decode(vals):
    """Mining values -> global candidate indices (-1 for pads/invalid)."""
    v64 = vals.astype(np.float64)
    jg = np.rint((v64 - np.floor(v64)) * 2048.0).astype(np.int64)
    slot = np.arange(vals.shape[1])
    core = slot // KEPT
    half = (slot % KEPT) // 8
    loc = half[None, :] * GRP + jg             # index within shard
    idx = core[None, :] * NSHARD + loc
    idx[(loc >= NSHARD) | (jg >= GRP)] = -1
    return idx


def kernel(outlayer, c, train_ill, k):
    import jax
    import jax.numpy as jnp

    k = int(k)
    outlayer = np.asarray(outlayer, np.float32)
    train_ill = np.asarray(train_ill)
    X = np.ascontiguousarray(
        outlayer.transpose(1, 0, 2).reshape(N_, KD)).astype(np.float32)
    left = train_ill[:, 0].astype(np.int64)
    right = train_ill[:, 1].astype(np.int64)

    qidx = np.concatenate([right, left])
    uq, inv = np.unique(qidx, return_inverse=True)
    nu = uq.shape[0]
    nqt_total = max(1, -(-nu // QT))
    nq_pad = nqt_total * QT

    in_maps = _prep_inputs(X, uq, nq_pad)
    vals = _mine(in_maps, nqt_total,
                 trace=bool(int(os.environ.get("KNN_TRACE", "0"))))
    cand = _decode(vals[:nu])                  # [nu, 128]
    # self is injected explicitly (its psum value overflows the exact
    # fraction range); drop any decoded copy first
    cand[cand == uq[:, None]] = -1
    cand = np.concatenate([cand, uq[:, None]], axis=1)

    # exact rerank of kept candidates (fp32 dots, fp64 refine of the top),
    # pinned to the CPU backend (the neuron backend can't take this jit)
    X64 = X.astype(np.float64)
    sq = (X64 ** 2).sum(1)
    nkeep = k + 1
    candc = np.clip(cand, 0, N_ - 1)
    bad = cand < 0
    topd = np.empty((nu, nkeep), np.float64)
    B = 512
    margin = min(nkeep + 6, cand.shape[1])
    with jax.default_device(jax.devices("cpu")[0]):
        Xj = jnp.asarray(X)
        for s in range(0, nu, B):
            e = min(s + B, nu)
            q = uq[s:e]
            cb = candc[s:e]
            dot = np.array(jnp.einsum('bd,bkd->bk', Xj[q], Xj[cb]))
            dist = sq[q][:, None] + sq[cb] - 2.0 * dot.astype(np.float64)
            dist[bad[s:e]] = 1e30
            o = np.argpartition(dist, margin - 1, axis=1)[:, :margin]
            ci = np.take_along_axis(cb, o, axis=1)
            dv = ((X64[q][:, None, :] - X64[ci]) ** 2).sum(2)
            dv[np.take_along_axis(bad[s:e], o, axis=1)] = 1e30
            oo = np.argsort(dv, axis=1)[:, :nkeep]
            topd[s:e] = np.take_along_axis(dv, oo, axis=1)

    topd_full = topd[inv]                      # expand to query instances
    s_right = topd_full[:T_]                   # mining of right idx
    s_left = topd_full[T_:2 * T_]              # mining of left idx
    B2 = s_right[:, 1:]                        # drop self (col 0)
    B1 = s_left[:, 1:]
    D = ((X64[left] - X64[right]) ** 2).sum(1) + 1.0
    L1 = np.maximum(D[:, None] - B1, 0.0)
    L2 = np.maximum(D[:, None] - B2, 0.0)
    loss = (L1.mean() + L2.mean()) / 2.0
    return np.asarray(loss, dtype=np.float32)
